# revision 40
# baseline (speedup 1.0000x reference)
"""ContrastiveTokenLoss on Trainium2 (8 NeuronCores, Bass/Tile).

Problem (hardcoded): input [2, 2048, 32000] f32 logits, target [2, 2048] int.
ct_len = round(2048*0.25) = 512, win = round(512*0.5) = 256,
IGNORE_INDEX = -100, PAD_ID = 0.

loss = sum_{b, i<512} valid(b,i) * log1p( sum_{j in [i-256, i), tgt[b,j]!=0}
           exp(x[b,i,tgt[b,j]] - x[b,i,tgt_safe[b,i]]) ) / max(#valid, 1)

Sharding: the 512 contrastive positions are split across the 8 cores (64 per
core per batch element; positions >= 512 are never touched).  Each core
receives its slab of logits laid out vocab-major ([32001, 128]: the one vocab
row the loss needs per window token is then contiguous; row 32000 is a -1e9
sentinel that PAD window tokens are redirected to, which zeroes their exp
contribution), the per-chunk gather row indices, its own 128 targets, and one
constant table (additive -1e9 band mask | 128x128 identity).

On-device per core: 5 (6 when PAD/ignore tokens are present) indirect DMAs
gather the window rows the loss touches (512 B contiguous each, ~320 KB
instead of the 16 MB slab), PE transposes each [128,128] chunk back to
(position-partition, window-free) layout, DVE adds the band mask straight
out of PSUM and extracts the positive logit from the window diagonal, ACT
computes a fused exp+row-sum with -pos as a per-partition bias, then log1p
and one [128,2]x[128,1] matmul reduce loss / valid-count over partitions.
Each core returns [loss_sum, valid_count]; the host sums the 8 partials and
divides.
"""

import numpy as np
from contextlib import ExitStack

import concourse.bass as bass
import concourse.bacc as bacc
import concourse.mybir as mybir
import concourse.tile as tile
from concourse.bass_utils import run_bass_kernel_spmd

B, T, V = 2, 2048, 32000
CT = 512
WIN = 256
IGNORE_INDEX = -100
PAD_ID = 0
NCORES = 8
CI = CT // NCORES          # 64 positions per core per batch
W = WIN + CI               # 320 window positions per core per batch
NW = B * W                 # 640 window rows per core
P = B * CI                 # 128 partition rows = (batch, local position)
F32 = mybir.dt.float32
I32 = mybir.dt.int32

_CACHE = {}


def _build(pos_chunk):
    """pos_chunk=False: 5 gathers, pos read off the window diagonal (exact
    when every target in [0, CT) is > 0, which the host checks).
    pos_chunk=True: an extra leading gather chunk holds the own-target rows
    unsentineled so PAD own-targets still produce the right pos."""
    nch = 6 if pos_chunk else 5
    nc = bacc.Bacc("TRN2", target_bir_lowering=False)
    xt = nc.dram_tensor("xt", [V + 1, P], F32, kind="ExternalInput")
    idx = nc.dram_tensor("idx", [P, nch], I32, kind="ExternalInput")
    to = nc.dram_tensor("to", [P, 1], I32, kind="ExternalInput")
    cst = nc.dram_tensor("cst", [P, NW + P], F32, kind="ExternalInput")
    out = nc.dram_tensor("out", [2, 1], F32, kind="ExternalOutput")

    with ExitStack() as ctx:
        tc = ctx.enter_context(tile.TileContext(nc))
        sb = ctx.enter_context(tc.tile_pool(name="sb", bufs=1))
        ps = ctx.enter_context(tc.tile_pool(name="ps", bufs=1, space="PSUM"))

        # gather row indices first (host pre-clamped and pre-sentineled) so
        # the gathers start as early as possible
        it = sb.tile([P, nch], I32)
        nc.sync.dma_start(it[:], idx[:])

        cst_sb = sb.tile([P, NW + P], F32)
        nc.sync.dma_start(cst_sb[:], cst[:])
        mb = cst_sb[:, 0:NW]
        ident = cst_sb[:, NW : NW + P]
        to_sb = sb.tile([P, 1], I32)
        nc.sync.dma_start(to_sb[:], to[:])

        # valid flags (lv[:, 1]) and, for the ln(valid*r + inv) fold, the
        # inverted flag — both off the critical path
        lv = sb.tile([P, 2], F32)
        nc.vector.tensor_scalar(
            lv[:, 1:2], to_sb[:], IGNORE_INDEX, None, mybir.AluOpType.not_equal
        )
        inv = sb.tile([P, 1], F32)
        nc.vector.tensor_scalar(
            inv[:], to_sb[:], IGNORE_INDEX, None, mybir.AluOpType.is_equal
        )

        # Preload the additive band mask into each window chunk's PSUM bank
        # (psum_c = I.T @ mb_c = mb_c) before the gathers land; the transpose
        # then ACCUMULATES gt.T on top (one matmul, start=False), so exp can
        # read (gt.T + mb) straight from PSUM with no DVE add.  In the
        # 5-chunk variant the mask keeps the window diagonal (own token) at 0
        # so its exp term is exactly 1 == log1p's "+1".
        order = [0, 1, 2, 3, 4, 5] if pos_chunk else [2, 4, 0, 1, 3]
        pts = {}
        for c in order:
            pt = ps.tile([P, P], F32, tag=f"pt{c}", space="PSUM")
            pts[c] = pt
            if not (pos_chunk and c == 0):
                # mask blocks are stored pre-transposed, so the preload runs
                # on the fast transpose datapath: pt = (mbT_c).T = mb_c
                w = c - 1 if pos_chunk else c
                sl = slice(w * P, (w + 1) * P)
                nc.tensor.matmul(
                    out=pt[:], lhsT=mb[:, sl], rhs=ident,
                    is_transpose=True, start=True, stop=False,
                )

        pd = sb.tile([P, P], F32)
        npos = sb.tile([P, 1], F32)
        e = sb.tile([P, NW], F32)
        r5 = sb.tile([P, nch], F32)
        h = P // B
        for c in order:
            gt = sb.tile([P, P], F32, tag=f"gt{c}")
            nc.gpsimd.indirect_dma_start(
                out=gt[:],
                out_offset=None,
                in_=xt[:],
                in_offset=bass.IndirectOffsetOnAxis(ap=it[:, c : c + 1], axis=0),
            )
            pt = pts[c]
            if pos_chunk and c == 0:
                # chunk 0 rows are the own targets: pos[p] = gt.T[p, p]
                nc.tensor.transpose(out=pt[:], in_=gt[:], identity=ident)
                nc.vector.tensor_tensor(pd[:], pt[:], ident, mybir.AluOpType.mult)
            else:
                # pt += gt.T on the transpose datapath
                nc.tensor.matmul(
                    out=pt[:], lhsT=gt[:], rhs=ident,
                    is_transpose=True, start=False, stop=True,
                )
            if (pos_chunk and c == 0) or (not pos_chunk and c == 4):
                # last diag-carrying chunk landed: finish pos (diag mask mb
                # is 0 there in the 5-chunk variant, so psum diag == gt.T)
                if not pos_chunk:
                    nc.vector.tensor_tensor(
                        pd[0:h, :], pts[2][0:h, :], ident[0:h, :],
                        mybir.AluOpType.mult,
                    )
                    nc.vector.tensor_tensor(
                        pd[h:P, :], pts[4][h:P, :], ident[h:P, :],
                        mybir.AluOpType.mult,
                    )
                nc.vector.reduce_sum(
                    npos[:], pd[:], axis=mybir.AxisListType.X, negate=True
                )
        # fused exp(chunk - pos) with per-row accumulation, read from PSUM;
        # each exp fires as soon as its chunk and npos are ready
        for c in order:
            if pos_chunk and c == 0:
                continue
            w = c - 1 if pos_chunk else c
            sl = slice(w * P, (w + 1) * P)
            nc.scalar.activation(
                e[:, sl], pts[c][:], mybir.ActivationFunctionType.Exp,
                bias=npos[:], scale=1.0, accum_out=r5[:, c : c + 1],
            )
        r = sb.tile([P, 1], F32)
        if pos_chunk:
            nc.vector.reduce_sum(r[:], r5[:, 1:], axis=mybir.AxisListType.X)
        else:
            nc.vector.reduce_sum(r[:], r5[:], axis=mybir.AxisListType.X)

        # lv[:, 0] = valid * log1p(sum of masked exp):
        #   5-chunk: r already contains the diagonal's exp(0) == 1, so
        #            ln(valid*r + inv) = log1p(.) when valid else ln(1) = 0
        #   6-chunk: ln(valid*r + 1) = log1p(r) when valid else 0
        if pos_chunk:
            nc.scalar.activation(
                lv[:, 0:1], r[:], mybir.ActivationFunctionType.Ln,
                bias=1.0, scale=lv[:, 1:2],
            )
        else:
            nc.scalar.activation(
                lv[:, 0:1], r[:], mybir.ActivationFunctionType.Ln,
                bias=inv[:], scale=lv[:, 1:2],
            )

        # partition reduction: out[2, 1] = lv.T @ ones
        ones = sb.tile([P, 1], F32)
        nc.vector.memset(ones[:], 1.0)
        acc = ps.tile([2, 1], F32, space="PSUM")
        nc.tensor.matmul(out=acc[:], lhsT=lv[:], rhs=ones[:], start=True, stop=True)
        res = sb.tile([2, 1], F32)
        nc.vector.tensor_copy(res[:], acc[:])
        nc.sync.dma_start(out[:], res[:])
    nc.compile()
    return nc


def _get_nc(pos_chunk):
    key = f"nc{pos_chunk}"
    if key not in _CACHE:
        _CACHE[key] = _build(pos_chunk)
    return _CACHE[key]


def _consts():
    if "consts" not in _CACHE:
        p = np.arange(P, dtype=np.int64)
        il = (p % CI)[:, None]
        bp = (p // CI)[:, None]
        f = np.arange(NW, dtype=np.int64)[None, :]
        jl = f % W
        bf = f // W
        band = (bf == bp) & (jl >= il) & (jl < il + WIN)
        cstv = np.full((P, NW + P), -1e9, np.float32)
        cstv[:, 0:NW][band] = 0.0
        cstv[:, NW:] = np.eye(P, dtype=np.float32)
        # 5-chunk variant: keep each partition's own window-diagonal entry
        # unmasked — its exp term is exactly 1 and plays log1p's "+1"
        cstv5 = cstv.copy()
        p_i = np.arange(P)
        cstv5[p_i, (p_i // CI) * W + WIN + (p_i % CI)] = 0.0
        # store the window mask blocks transposed: the kernel preloads each
        # into PSUM via the transpose datapath (pt = block.T)
        for cv in (cstv, cstv5):
            for c in range(NW // P):
                sl = slice(c * P, (c + 1) * P)
                cv[:, sl] = cv[:, sl].T.copy()
        # diag positions in the flat window: j == i rows per batch
        diagf = np.zeros(NW, bool)
        diagf[WIN : W] = True
        diagf[W + WIN : 2 * W] = True
        _CACHE["consts"] = (
            np.ascontiguousarray(cstv),
            np.ascontiguousarray(cstv5),
            diagf,
        )
    return _CACHE["consts"]


def kernel(input, target, _trace=False):
    input = np.asarray(input, dtype=np.float32)
    target = np.asarray(target)
    cstv6, cstv5, diagf = _consts()
    t32 = target[:, :CT].astype(np.int32)

    # fast path: pos can be read off the window diagonal iff no target in the
    # contrastive range is PAD (0) or negative
    pos_chunk = bool((t32 <= 0).any())

    in_maps = []
    for k in range(NCORES):
        s = k * CI
        lo = s - WIN
        if lo >= 0:
            twk = t32[:, lo : s + CI]
        else:
            twk = np.concatenate(
                [np.zeros((B, -lo), np.int32), t32[:, : s + CI]], axis=1
            )
        tok = t32[:, s : s + CI].reshape(-1)
        win_ids = twk.reshape(-1)
        # tgt_safe clamp + redirect PAD window tokens to the sentinel row V
        # (the own-target/diagonal copies stay clamped so pos is exact)
        safe = np.maximum(win_ids, 0)
        sent_ids = np.where(win_ids == PAD_ID, V, safe)
        if pos_chunk:
            ids_flat = np.concatenate([np.maximum(tok, 0), sent_ids])
        else:
            ids_flat = np.where(diagf, safe, sent_ids)
        nch = ids_flat.size // P
        idxs = np.ascontiguousarray(ids_flat.reshape(nch, P).T)
        xtk = np.empty((V + 1, P), np.float32)
        xtk[:V] = input[:, s : s + CI, :].reshape(P, V).T
        xtk[V:] = -1e9
        in_maps.append(
            {
                "xt": xtk,
                "idx": idxs,
                "to": np.ascontiguousarray(tok).reshape(P, 1),
                "cst": cstv6 if pos_chunk else cstv5,
            }
        )

    nc = _get_nc(pos_chunk)
    br = run_bass_kernel_spmd(
        nc, in_maps, core_ids=list(range(NCORES)), trace=_trace
    )
    rs = np.stack([r["out"] for r in br.results])  # [8, 2, 1]
    loss_sum = rs[:, 0, 0].astype(np.float64).sum()
    cnt = rs[:, 1, 0].astype(np.float64).sum()
    kernel.last_results = br
    return np.asarray(np.float32(loss_sum / max(cnt, 1.0)))


# revision 42
# speedup vs baseline: 1.0438x; 1.0438x over previous
"""ContrastiveTokenLoss on Trainium2 (8 NeuronCores, Bass/Tile).

Problem (hardcoded): input [2, 2048, 32000] f32 logits, target [2, 2048] int.
ct_len = round(2048*0.25) = 512, win = round(512*0.5) = 256,
IGNORE_INDEX = -100, PAD_ID = 0.

loss = sum_{b, i<512} valid(b,i) * log1p( sum_{j in [i-256, i), tgt[b,j]!=0}
           exp(x[b,i,tgt[b,j]] - x[b,i,tgt_safe[b,i]]) ) / max(#valid, 1)

Sharding: the 512 contrastive positions are split across the 8 cores (64 per
core per batch element; positions >= 512 are never touched).  Each core
receives its slab of logits laid out vocab-major ([32001, 128]: the one vocab
row the loss needs per window token is then contiguous; row 32000 is a -1e9
sentinel that PAD window tokens are redirected to, which zeroes their exp
contribution), the per-chunk gather row indices, its own 128 targets, and one
constant table (additive -1e9 band mask | 128x128 identity).

On-device per core: 5 (6 when PAD/ignore tokens are present) indirect DMAs
gather the window rows the loss touches (512 B contiguous each, ~320 KB
instead of the 16 MB slab), PE transposes each [128,128] chunk back to
(position-partition, window-free) layout, DVE adds the band mask straight
out of PSUM and extracts the positive logit from the window diagonal, ACT
computes a fused exp+row-sum with -pos as a per-partition bias, then log1p
and one [128,2]x[128,1] matmul reduce loss / valid-count over partitions.
Each core returns [loss_sum, valid_count]; the host sums the 8 partials and
divides.
"""

import numpy as np
from contextlib import ExitStack

import concourse.bass as bass
import concourse.bacc as bacc
import concourse.mybir as mybir
import concourse.tile as tile
from concourse.bass_utils import run_bass_kernel_spmd

B, T, V = 2, 2048, 32000
CT = 512
WIN = 256
IGNORE_INDEX = -100
PAD_ID = 0
NCORES = 8
CI = CT // NCORES          # 64 positions per core per batch
W = WIN + CI               # 320 window positions per core per batch
NW = B * W                 # 640 window rows per core
P = B * CI                 # 128 partition rows = (batch, local position)
F32 = mybir.dt.float32
I32 = mybir.dt.int32

_CACHE = {}


def _build(pos_chunk):
    """pos_chunk=False: 5 gathers, pos read off the window diagonal (exact
    when every target in [0, CT) is > 0, which the host checks).
    pos_chunk=True: an extra leading gather chunk holds the own-target rows
    unsentineled so PAD own-targets still produce the right pos."""
    nch = 6 if pos_chunk else 5
    nc = bacc.Bacc("TRN2", target_bir_lowering=False)
    xt = nc.dram_tensor("xt", [V + 1, P], F32, kind="ExternalInput")
    idx = nc.dram_tensor("idx", [P, nch], I32, kind="ExternalInput")
    to = nc.dram_tensor("to", [P, 1], I32, kind="ExternalInput")
    cst = nc.dram_tensor("cst", [P, NW + P], F32, kind="ExternalInput")
    out = nc.dram_tensor("out", [2, 1], F32, kind="ExternalOutput")

    with ExitStack() as ctx:
        tc = ctx.enter_context(tile.TileContext(nc))
        sb = ctx.enter_context(tc.tile_pool(name="sb", bufs=1))
        ps = ctx.enter_context(tc.tile_pool(name="ps", bufs=1, space="PSUM"))

        # gather row indices first (host pre-clamped and pre-sentineled) so
        # the gathers start as early as possible
        # scalar (ACT) HWDGE queue: issues ahead of the Sync-queue drain and
        # the constant DMAs, so the gathers start as early as possible
        it = sb.tile([P, nch], I32)
        nc.scalar.dma_start(it[:], idx[:])

        cst_sb = sb.tile([P, NW + P], F32)
        nc.sync.dma_start(cst_sb[:], cst[:])
        mb = cst_sb[:, 0:NW]
        ident = cst_sb[:, NW : NW + P]
        to_sb = sb.tile([P, 1], I32)
        nc.sync.dma_start(to_sb[:], to[:])

        # valid flags (lv[:, 1]) and, for the ln(valid*r + inv) fold, the
        # inverted flag — both off the critical path
        lv = sb.tile([P, 2], F32)
        nc.vector.tensor_scalar(
            lv[:, 1:2], to_sb[:], IGNORE_INDEX, None, mybir.AluOpType.not_equal
        )
        inv = sb.tile([P, 1], F32)
        nc.vector.tensor_scalar(
            inv[:], to_sb[:], IGNORE_INDEX, None, mybir.AluOpType.is_equal
        )

        # Preload the additive band mask into each window chunk's PSUM bank
        # (psum_c = I.T @ mb_c = mb_c) before the gathers land; the transpose
        # then ACCUMULATES gt.T on top (one matmul, start=False), so exp can
        # read (gt.T + mb) straight from PSUM with no DVE add.  In the
        # 5-chunk variant the mask keeps the window diagonal (own token) at 0
        # so its exp term is exactly 1 == log1p's "+1".
        order = [0, 1, 2, 3, 4, 5] if pos_chunk else [2, 4, 0, 1, 3]
        pts = {}
        for c in order:
            pt = ps.tile([P, P], F32, tag=f"pt{c}", space="PSUM")
            pts[c] = pt
            if not (pos_chunk and c == 0):
                # mask blocks are stored pre-transposed, so the preload runs
                # on the fast transpose datapath: pt = (mbT_c).T = mb_c
                w = c - 1 if pos_chunk else c
                sl = slice(w * P, (w + 1) * P)
                nc.tensor.matmul(
                    out=pt[:], lhsT=mb[:, sl], rhs=ident,
                    is_transpose=True, start=True, stop=False,
                )

        pd = sb.tile([P, P], F32)
        npos = sb.tile([P, 1], F32)
        e = sb.tile([P, NW], F32)
        r5 = sb.tile([P, nch], F32)
        h = P // B
        for c in order:
            gt = sb.tile([P, P], F32, tag=f"gt{c}")
            nc.gpsimd.indirect_dma_start(
                out=gt[:],
                out_offset=None,
                in_=xt[:],
                in_offset=bass.IndirectOffsetOnAxis(ap=it[:, c : c + 1], axis=0),
            )
            pt = pts[c]
            if pos_chunk and c == 0:
                # chunk 0 rows are the own targets: pos[p] = gt.T[p, p]
                nc.tensor.transpose(out=pt[:], in_=gt[:], identity=ident)
                nc.vector.tensor_tensor(pd[:], pt[:], ident, mybir.AluOpType.mult)
            else:
                # pt += gt.T on the transpose datapath
                nc.tensor.matmul(
                    out=pt[:], lhsT=gt[:], rhs=ident,
                    is_transpose=True, start=False, stop=True,
                )
            if (pos_chunk and c == 0) or (not pos_chunk and c == 4):
                # last diag-carrying chunk landed: finish pos (diag mask mb
                # is 0 there in the 5-chunk variant, so psum diag == gt.T)
                if not pos_chunk:
                    nc.vector.tensor_tensor(
                        pd[0:h, :], pts[2][0:h, :], ident[0:h, :],
                        mybir.AluOpType.mult,
                    )
                    nc.vector.tensor_tensor(
                        pd[h:P, :], pts[4][h:P, :], ident[h:P, :],
                        mybir.AluOpType.mult,
                    )
                nc.vector.reduce_sum(
                    npos[:], pd[:], axis=mybir.AxisListType.X, negate=True
                )
        # fused exp(chunk - pos) with per-row accumulation, read from PSUM;
        # each exp fires as soon as its chunk and npos are ready.  Window
        # chunks 0/1 only hold batch-0 tokens and 3/4 only batch-1 (the
        # other partition half is -1e9-masked), so those exps run half-width.
        nc.vector.memset(r5[:], 0.0)
        for c in order:
            if pos_chunk and c == 0:
                continue
            w = c - 1 if pos_chunk else c
            sl = slice(w * P, (w + 1) * P)
            if w in (0, 1):
                pr = slice(0, h)
            elif w in (3, 4):
                pr = slice(h, P)
            else:
                pr = slice(0, P)
            nc.scalar.activation(
                e[pr, sl], pts[c][pr, :], mybir.ActivationFunctionType.Exp,
                bias=npos[pr, :], scale=1.0, accum_out=r5[pr, c : c + 1],
            )
        r = sb.tile([P, 1], F32)
        if pos_chunk:
            nc.vector.reduce_sum(r[:], r5[:, 1:], axis=mybir.AxisListType.X)
        else:
            nc.vector.reduce_sum(r[:], r5[:], axis=mybir.AxisListType.X)

        # lv[:, 0] = valid * log1p(sum of masked exp):
        #   5-chunk: r already contains the diagonal's exp(0) == 1, so
        #            ln(valid*r + inv) = log1p(.) when valid else ln(1) = 0
        #   6-chunk: ln(valid*r + 1) = log1p(r) when valid else 0
        if pos_chunk:
            nc.scalar.activation(
                lv[:, 0:1], r[:], mybir.ActivationFunctionType.Ln,
                bias=1.0, scale=lv[:, 1:2],
            )
        else:
            nc.scalar.activation(
                lv[:, 0:1], r[:], mybir.ActivationFunctionType.Ln,
                bias=inv[:], scale=lv[:, 1:2],
            )

        # partition reduction: out[2, 1] = lv.T @ ones
        ones = sb.tile([P, 1], F32)
        nc.vector.memset(ones[:], 1.0)
        acc = ps.tile([2, 1], F32, space="PSUM")
        nc.tensor.matmul(out=acc[:], lhsT=lv[:], rhs=ones[:], start=True, stop=True)
        res = sb.tile([2, 1], F32)
        nc.vector.tensor_copy(res[:], acc[:])
        nc.sync.dma_start(out[:], res[:])
    nc.compile()
    return nc


def _get_nc(pos_chunk):
    key = f"nc{pos_chunk}"
    if key not in _CACHE:
        _CACHE[key] = _build(pos_chunk)
    return _CACHE[key]


def _consts():
    if "consts" not in _CACHE:
        p = np.arange(P, dtype=np.int64)
        il = (p % CI)[:, None]
        bp = (p // CI)[:, None]
        f = np.arange(NW, dtype=np.int64)[None, :]
        jl = f % W
        bf = f // W
        band = (bf == bp) & (jl >= il) & (jl < il + WIN)
        cstv = np.full((P, NW + P), -1e9, np.float32)
        cstv[:, 0:NW][band] = 0.0
        cstv[:, NW:] = np.eye(P, dtype=np.float32)
        # 5-chunk variant: keep each partition's own window-diagonal entry
        # unmasked — its exp term is exactly 1 and plays log1p's "+1"
        cstv5 = cstv.copy()
        p_i = np.arange(P)
        cstv5[p_i, (p_i // CI) * W + WIN + (p_i % CI)] = 0.0
        # store the window mask blocks transposed: the kernel preloads each
        # into PSUM via the transpose datapath (pt = block.T)
        for cv in (cstv, cstv5):
            for c in range(NW // P):
                sl = slice(c * P, (c + 1) * P)
                cv[:, sl] = cv[:, sl].T.copy()
        # diag positions in the flat window: j == i rows per batch
        diagf = np.zeros(NW, bool)
        diagf[WIN : W] = True
        diagf[W + WIN : 2 * W] = True
        _CACHE["consts"] = (
            np.ascontiguousarray(cstv),
            np.ascontiguousarray(cstv5),
            diagf,
        )
    return _CACHE["consts"]


def kernel(input, target, _trace=False):
    input = np.asarray(input, dtype=np.float32)
    target = np.asarray(target)
    cstv6, cstv5, diagf = _consts()
    t32 = target[:, :CT].astype(np.int32)

    # fast path: pos can be read off the window diagonal iff no target in the
    # contrastive range is PAD (0) or negative
    pos_chunk = bool((t32 <= 0).any())

    in_maps = []
    for k in range(NCORES):
        s = k * CI
        lo = s - WIN
        if lo >= 0:
            twk = t32[:, lo : s + CI]
        else:
            twk = np.concatenate(
                [np.zeros((B, -lo), np.int32), t32[:, : s + CI]], axis=1
            )
        tok = t32[:, s : s + CI].reshape(-1)
        win_ids = twk.reshape(-1)
        # tgt_safe clamp + redirect PAD window tokens to the sentinel row V
        # (the own-target/diagonal copies stay clamped so pos is exact)
        safe = np.maximum(win_ids, 0)
        sent_ids = np.where(win_ids == PAD_ID, V, safe)
        if pos_chunk:
            ids_flat = np.concatenate([np.maximum(tok, 0), sent_ids])
        else:
            ids_flat = np.where(diagf, safe, sent_ids)
        nch = ids_flat.size // P
        idxs = np.ascontiguousarray(ids_flat.reshape(nch, P).T)
        xtk = np.empty((V + 1, P), np.float32)
        xtk[:V] = input[:, s : s + CI, :].reshape(P, V).T
        xtk[V:] = -1e9
        in_maps.append(
            {
                "xt": xtk,
                "idx": idxs,
                "to": np.ascontiguousarray(tok).reshape(P, 1),
                "cst": cstv6 if pos_chunk else cstv5,
            }
        )

    nc = _get_nc(pos_chunk)
    br = run_bass_kernel_spmd(
        nc, in_maps, core_ids=list(range(NCORES)), trace=_trace
    )
    rs = np.stack([r["out"] for r in br.results])  # [8, 2, 1]
    loss_sum = rs[:, 0, 0].astype(np.float64).sum()
    cnt = rs[:, 1, 0].astype(np.float64).sum()
    kernel.last_results = br
    return np.asarray(np.float32(loss_sum / max(cnt, 1.0)))


# revision 43
# speedup vs baseline: 1.0529x; 1.0087x over previous
"""ContrastiveTokenLoss on Trainium2 (8 NeuronCores, Bass/Tile).

Problem (hardcoded): input [2, 2048, 32000] f32 logits, target [2, 2048] int.
ct_len = round(2048*0.25) = 512, win = round(512*0.5) = 256,
IGNORE_INDEX = -100, PAD_ID = 0.

loss = sum_{b, i<512} valid(b,i) * log1p( sum_{j in [i-256, i), tgt[b,j]!=0}
           exp(x[b,i,tgt[b,j]] - x[b,i,tgt_safe[b,i]]) ) / max(#valid, 1)

Sharding: the 512 contrastive positions are split across the 8 cores (64 per
core per batch element; positions >= 512 are never touched).  Each core
receives its slab of logits laid out vocab-major ([32001, 128]: the one vocab
row the loss needs per window token is then contiguous; row 32000 is a -1e9
sentinel that PAD window tokens are redirected to, which zeroes their exp
contribution), the per-chunk gather row indices, its own 128 targets, and one
constant table (additive -1e9 band mask | 128x128 identity).

On-device per core: 5 (6 when PAD/ignore tokens are present) indirect DMAs
gather the window rows the loss touches (512 B contiguous each, ~320 KB
instead of the 16 MB slab), PE transposes each [128,128] chunk back to
(position-partition, window-free) layout, DVE adds the band mask straight
out of PSUM and extracts the positive logit from the window diagonal, ACT
computes a fused exp+row-sum with -pos as a per-partition bias, then log1p
and one [128,2]x[128,1] matmul reduce loss / valid-count over partitions.
Each core returns [loss_sum, valid_count]; the host sums the 8 partials and
divides.
"""

import numpy as np
from contextlib import ExitStack

import concourse.bass as bass
import concourse.bacc as bacc
import concourse.mybir as mybir
import concourse.tile as tile
from concourse.bass_utils import run_bass_kernel_spmd

B, T, V = 2, 2048, 32000
CT = 512
WIN = 256
IGNORE_INDEX = -100
PAD_ID = 0
NCORES = 8
CI = CT // NCORES          # 64 positions per core per batch
W = WIN + CI               # 320 window positions per core per batch
NW = B * W                 # 640 window rows per core
P = B * CI                 # 128 partition rows = (batch, local position)
F32 = mybir.dt.float32
I32 = mybir.dt.int32

_CACHE = {}


def _build(pos_chunk):
    """pos_chunk=False: 5 gathers, pos read off the window diagonal (exact
    when every target in [0, CT) is > 0, which the host checks).
    pos_chunk=True: an extra leading gather chunk holds the own-target rows
    unsentineled so PAD own-targets still produce the right pos."""
    nch = 6 if pos_chunk else 5
    nc = bacc.Bacc("TRN2", target_bir_lowering=False)
    xt = nc.dram_tensor("xt", [V + 1, P], F32, kind="ExternalInput")
    idx = nc.dram_tensor("idx", [P, nch], I32, kind="ExternalInput")
    to = nc.dram_tensor("to", [P, 1], I32, kind="ExternalInput")
    cst = nc.dram_tensor("cst", [P, NW + P], F32, kind="ExternalInput")
    out = nc.dram_tensor("out", [2, 1], F32, kind="ExternalOutput")

    with ExitStack() as ctx:
        tc = ctx.enter_context(tile.TileContext(nc))
        sb = ctx.enter_context(tc.tile_pool(name="sb", bufs=1))
        ps = ctx.enter_context(tc.tile_pool(name="ps", bufs=1, space="PSUM"))

        # gather row indices first (host pre-clamped and pre-sentineled) so
        # the gathers start as early as possible
        it = sb.tile([P, nch], I32)
        nc.sync.dma_start(it[:], idx[:])

        cst_sb = sb.tile([P, NW + P], F32)
        nc.sync.dma_start(cst_sb[:], cst[:])
        mb = cst_sb[:, 0:NW]
        ident = cst_sb[:, NW : NW + P]
        to_sb = sb.tile([P, 1], I32)
        nc.sync.dma_start(to_sb[:], to[:])

        # valid flags (lv[:, 1]) and, for the ln(valid*r + inv) fold, the
        # inverted flag — both off the critical path
        lv = sb.tile([P, 2], F32)
        nc.vector.tensor_scalar(
            lv[:, 1:2], to_sb[:], IGNORE_INDEX, None, mybir.AluOpType.not_equal
        )
        inv = sb.tile([P, 1], F32)
        nc.vector.tensor_scalar(
            inv[:], to_sb[:], IGNORE_INDEX, None, mybir.AluOpType.is_equal
        )

        # Preload the additive band mask into each window chunk's PSUM bank
        # (psum_c = I.T @ mb_c = mb_c) before the gathers land; the transpose
        # then ACCUMULATES gt.T on top (one matmul, start=False), so exp can
        # read (gt.T + mb) straight from PSUM with no DVE add.  In the
        # 5-chunk variant the mask keeps the window diagonal (own token) at 0
        # so its exp term is exactly 1 == log1p's "+1".
        order = [0, 1, 2, 3, 4, 5] if pos_chunk else [2, 4, 0, 1, 3]
        pts = {}
        for c in order:
            pt = ps.tile([P, P], F32, tag=f"pt{c}", space="PSUM")
            pts[c] = pt
            if not (pos_chunk and c == 0):
                # mask blocks are stored pre-transposed, so the preload runs
                # on the fast transpose datapath: pt = (mbT_c).T = mb_c
                w = c - 1 if pos_chunk else c
                sl = slice(w * P, (w + 1) * P)
                nc.tensor.matmul(
                    out=pt[:], lhsT=mb[:, sl], rhs=ident,
                    is_transpose=True, start=True, stop=False,
                )

        pd = sb.tile([P, P], F32)
        npos = sb.tile([P, 1], F32)
        e = sb.tile([P, NW], F32)
        r5 = sb.tile([P, nch], F32)
        h = P // B
        for c in order:
            gt = sb.tile([P, P], F32, tag=f"gt{c}")
            nc.gpsimd.indirect_dma_start(
                out=gt[:],
                out_offset=None,
                in_=xt[:],
                in_offset=bass.IndirectOffsetOnAxis(ap=it[:, c : c + 1], axis=0),
            )
            pt = pts[c]
            if pos_chunk and c == 0:
                # chunk 0 rows are the own targets: pos[p] = gt.T[p, p]
                nc.tensor.transpose(out=pt[:], in_=gt[:], identity=ident)
                nc.vector.tensor_tensor(pd[:], pt[:], ident, mybir.AluOpType.mult)
            else:
                # pt += gt.T on the transpose datapath
                nc.tensor.matmul(
                    out=pt[:], lhsT=gt[:], rhs=ident,
                    is_transpose=True, start=False, stop=True,
                )
            if (pos_chunk and c == 0) or (not pos_chunk and c == 4):
                # last diag-carrying chunk landed: finish pos (diag mask mb
                # is 0 there in the 5-chunk variant, so psum diag == gt.T)
                if not pos_chunk:
                    nc.vector.tensor_tensor(
                        pd[0:h, :], pts[2][0:h, :], ident[0:h, :],
                        mybir.AluOpType.mult,
                    )
                    nc.vector.tensor_tensor(
                        pd[h:P, :], pts[4][h:P, :], ident[h:P, :],
                        mybir.AluOpType.mult,
                    )
                nc.vector.reduce_sum(
                    npos[:], pd[:], axis=mybir.AxisListType.X, negate=True
                )
        # fused exp(chunk - pos) with per-row accumulation, read from PSUM;
        # each exp fires as soon as its chunk and npos are ready.  Window
        # chunks 0/1 only hold batch-0 tokens and 3/4 only batch-1 (the
        # other partition half is -1e9-masked), so those exps run half-width.
        nc.vector.memset(r5[:], 0.0)
        for c in order:
            if pos_chunk and c == 0:
                continue
            w = c - 1 if pos_chunk else c
            sl = slice(w * P, (w + 1) * P)
            if w in (0, 1):
                pr = slice(0, h)
            elif w in (3, 4):
                pr = slice(h, P)
            else:
                pr = slice(0, P)
            nc.scalar.activation(
                e[pr, sl], pts[c][pr, :], mybir.ActivationFunctionType.Exp,
                bias=npos[pr, :], scale=1.0, accum_out=r5[pr, c : c + 1],
            )
        r = sb.tile([P, 1], F32)
        if pos_chunk:
            nc.vector.reduce_sum(r[:], r5[:, 1:], axis=mybir.AxisListType.X)
        else:
            nc.vector.reduce_sum(r[:], r5[:], axis=mybir.AxisListType.X)

        # lv[:, 0] = valid * log1p(sum of masked exp):
        #   5-chunk: r already contains the diagonal's exp(0) == 1, so
        #            ln(valid*r + inv) = log1p(.) when valid else ln(1) = 0
        #   6-chunk: ln(valid*r + 1) = log1p(r) when valid else 0
        if pos_chunk:
            nc.scalar.activation(
                lv[:, 0:1], r[:], mybir.ActivationFunctionType.Ln,
                bias=1.0, scale=lv[:, 1:2],
            )
        else:
            nc.scalar.activation(
                lv[:, 0:1], r[:], mybir.ActivationFunctionType.Ln,
                bias=inv[:], scale=lv[:, 1:2],
            )

        # partition reduction: out[2, 1] = lv.T @ ones
        ones = sb.tile([P, 1], F32)
        nc.vector.memset(ones[:], 1.0)
        acc = ps.tile([2, 1], F32, space="PSUM")
        nc.tensor.matmul(out=acc[:], lhsT=lv[:], rhs=ones[:], start=True, stop=True)
        res = sb.tile([2, 1], F32)
        nc.vector.tensor_copy(res[:], acc[:])
        nc.sync.dma_start(out[:], res[:])
    nc.compile()
    return nc


def _get_nc(pos_chunk):
    key = f"nc{pos_chunk}"
    if key not in _CACHE:
        _CACHE[key] = _build(pos_chunk)
    return _CACHE[key]


def _consts():
    if "consts" not in _CACHE:
        p = np.arange(P, dtype=np.int64)
        il = (p % CI)[:, None]
        bp = (p // CI)[:, None]
        f = np.arange(NW, dtype=np.int64)[None, :]
        jl = f % W
        bf = f // W
        band = (bf == bp) & (jl >= il) & (jl < il + WIN)
        cstv = np.full((P, NW + P), -1e9, np.float32)
        cstv[:, 0:NW][band] = 0.0
        cstv[:, NW:] = np.eye(P, dtype=np.float32)
        # 5-chunk variant: keep each partition's own window-diagonal entry
        # unmasked — its exp term is exactly 1 and plays log1p's "+1"
        cstv5 = cstv.copy()
        p_i = np.arange(P)
        cstv5[p_i, (p_i // CI) * W + WIN + (p_i % CI)] = 0.0
        # store the window mask blocks transposed: the kernel preloads each
        # into PSUM via the transpose datapath (pt = block.T)
        for cv in (cstv, cstv5):
            for c in range(NW // P):
                sl = slice(c * P, (c + 1) * P)
                cv[:, sl] = cv[:, sl].T.copy()
        # diag positions in the flat window: j == i rows per batch
        diagf = np.zeros(NW, bool)
        diagf[WIN : W] = True
        diagf[W + WIN : 2 * W] = True
        _CACHE["consts"] = (
            np.ascontiguousarray(cstv),
            np.ascontiguousarray(cstv5),
            diagf,
        )
    return _CACHE["consts"]


def kernel(input, target, _trace=False):
    input = np.asarray(input, dtype=np.float32)
    target = np.asarray(target)
    cstv6, cstv5, diagf = _consts()
    t32 = target[:, :CT].astype(np.int32)

    # fast path: pos can be read off the window diagonal iff no target in the
    # contrastive range is PAD (0) or negative
    pos_chunk = bool((t32 <= 0).any())

    in_maps = []
    for k in range(NCORES):
        s = k * CI
        lo = s - WIN
        if lo >= 0:
            twk = t32[:, lo : s + CI]
        else:
            twk = np.concatenate(
                [np.zeros((B, -lo), np.int32), t32[:, : s + CI]], axis=1
            )
        tok = t32[:, s : s + CI].reshape(-1)
        win_ids = twk.reshape(-1)
        # tgt_safe clamp + redirect PAD window tokens to the sentinel row V
        # (the own-target/diagonal copies stay clamped so pos is exact)
        safe = np.maximum(win_ids, 0)
        sent_ids = np.where(win_ids == PAD_ID, V, safe)
        if pos_chunk:
            ids_flat = np.concatenate([np.maximum(tok, 0), sent_ids])
        else:
            ids_flat = np.where(diagf, safe, sent_ids)
        nch = ids_flat.size // P
        idxs = np.ascontiguousarray(ids_flat.reshape(nch, P).T)
        xtk = np.empty((V + 1, P), np.float32)
        xtk[:V] = input[:, s : s + CI, :].reshape(P, V).T
        xtk[V:] = -1e9
        in_maps.append(
            {
                "xt": xtk,
                "idx": idxs,
                "to": np.ascontiguousarray(tok).reshape(P, 1),
                "cst": cstv6 if pos_chunk else cstv5,
            }
        )

    nc = _get_nc(pos_chunk)
    br = run_bass_kernel_spmd(
        nc, in_maps, core_ids=list(range(NCORES)), trace=_trace
    )
    rs = np.stack([r["out"] for r in br.results])  # [8, 2, 1]
    loss_sum = rs[:, 0, 0].astype(np.float64).sum()
    cnt = rs[:, 1, 0].astype(np.float64).sum()
    kernel.last_results = br
    return np.asarray(np.float32(loss_sum / max(cnt, 1.0)))


# revision 45
# speedup vs baseline: 1.0925x; 1.0376x over previous
"""ContrastiveTokenLoss on Trainium2 (8 NeuronCores, Bass/Tile).

Problem (hardcoded): input [2, 2048, 32000] f32 logits, target [2, 2048] int.
ct_len = round(2048*0.25) = 512, win = round(512*0.5) = 256,
IGNORE_INDEX = -100, PAD_ID = 0.

loss = sum_{b, i<512} valid(b,i) * log1p( sum_{j in [i-256, i), tgt[b,j]!=0}
           exp(x[b,i,tgt[b,j]] - x[b,i,tgt_safe[b,i]]) ) / max(#valid, 1)

Sharding: the 512 contrastive positions are split across the 8 cores (64 per
core per batch element; positions >= 512 are never touched).  Each core
receives its slab of logits laid out vocab-major ([32001, 128]: the one vocab
row the loss needs per window token is then contiguous; row 32000 is a -1e9
sentinel that PAD window tokens are redirected to, which zeroes their exp
contribution), the per-chunk gather row indices, its own 128 targets, and one
constant table (additive -1e9 band mask | 128x128 identity).

On-device per core: 5 (6 when PAD/ignore tokens are present) indirect DMAs
gather the window rows the loss touches (512 B contiguous each, ~320 KB
instead of the 16 MB slab), PE transposes each [128,128] chunk back to
(position-partition, window-free) layout, DVE adds the band mask straight
out of PSUM and extracts the positive logit from the window diagonal, ACT
computes a fused exp+row-sum with -pos as a per-partition bias, then log1p
and one [128,2]x[128,1] matmul reduce loss / valid-count over partitions.
Each core returns [loss_sum, valid_count]; the host sums the 8 partials and
divides.
"""

import numpy as np
from contextlib import ExitStack

import concourse.bass as bass
import concourse.bacc as bacc
import concourse.mybir as mybir
import concourse.tile as tile
from concourse.bass_utils import run_bass_kernel_spmd

B, T, V = 2, 2048, 32000
CT = 512
WIN = 256
IGNORE_INDEX = -100
PAD_ID = 0
NCORES = 8
CI = CT // NCORES          # 64 positions per core per batch
W = WIN + CI               # 320 window positions per core per batch
NW = B * W                 # 640 window rows per core
P = B * CI                 # 128 partition rows = (batch, local position)
F32 = mybir.dt.float32
I32 = mybir.dt.int32

_CACHE = {}


class _one_act_table:
    """While compiling, restrict Exp/Ln to the combined
    `natural_log_exp_and_others` ACT table set so the kernel needs a single
    table load instead of an Exp-set -> Ln-set swap on the critical path.
    Set ids are positional, so only set CONTENTS are filtered, never order."""

    def __enter__(self):
        self.orig = bacc.get_activation_tables

        def patched(arch):
            t = self.orig(arch)
            both = {
                mybir.ActivationFunctionType.Exp,
                mybir.ActivationFunctionType.Ln,
            }
            return {
                name: (fns if name == "natural_log_exp_and_others" else fns - both)
                for name, fns in t.items()
            }

        bacc.get_activation_tables = patched

    def __exit__(self, *a):
        bacc.get_activation_tables = self.orig


def _build(pos_chunk):
    """pos_chunk=False: 5 gathers, pos read off the window diagonal (exact
    when every target in [0, CT) is > 0, which the host checks).
    pos_chunk=True: an extra leading gather chunk holds the own-target rows
    unsentineled so PAD own-targets still produce the right pos."""
    nch = 6 if pos_chunk else 5
    nc = bacc.Bacc("TRN2", target_bir_lowering=False)
    xt = nc.dram_tensor("xt", [V + 1, P], F32, kind="ExternalInput")
    idx = nc.dram_tensor("idx", [P, nch], I32, kind="ExternalInput")
    to = nc.dram_tensor("to", [P, 1], I32, kind="ExternalInput")
    cst = nc.dram_tensor("cst", [P, NW + P], F32, kind="ExternalInput")
    out = nc.dram_tensor("out", [2, 1], F32, kind="ExternalOutput")

    with ExitStack() as ctx:
        tc = ctx.enter_context(tile.TileContext(nc))
        sb = ctx.enter_context(tc.tile_pool(name="sb", bufs=1))
        ps = ctx.enter_context(tc.tile_pool(name="ps", bufs=1, space="PSUM"))

        # gather row indices first (host pre-clamped and pre-sentineled) so
        # the gathers start as early as possible
        it = sb.tile([P, nch], I32)
        nc.sync.dma_start(it[:], idx[:])

        cst_sb = sb.tile([P, NW + P], F32)
        nc.sync.dma_start(cst_sb[:], cst[:])
        mb = cst_sb[:, 0:NW]
        ident = cst_sb[:, NW : NW + P]
        to_sb = sb.tile([P, 1], I32)
        nc.sync.dma_start(to_sb[:], to[:])

        # valid flags (lv[:, 1]) and, for the ln(valid*r + inv) fold, the
        # inverted flag — both off the critical path
        lv = sb.tile([P, 2], F32)
        nc.vector.tensor_scalar(
            lv[:, 1:2], to_sb[:], IGNORE_INDEX, None, mybir.AluOpType.not_equal
        )
        inv = sb.tile([P, 1], F32)
        nc.vector.tensor_scalar(
            inv[:], to_sb[:], IGNORE_INDEX, None, mybir.AluOpType.is_equal
        )

        # Preload the additive band mask into each window chunk's PSUM bank
        # (psum_c = I.T @ mb_c = mb_c) before the gathers land; the transpose
        # then ACCUMULATES gt.T on top (one matmul, start=False), so exp can
        # read (gt.T + mb) straight from PSUM with no DVE add.  In the
        # 5-chunk variant the mask keeps the window diagonal (own token) at 0
        # so its exp term is exactly 1 == log1p's "+1".
        order = [0, 1, 2, 3, 4, 5] if pos_chunk else [2, 4, 0, 1, 3]
        pts = {}
        for c in order:
            pt = ps.tile([P, P], F32, tag=f"pt{c}", space="PSUM")
            pts[c] = pt
            if not (pos_chunk and c == 0):
                # mask blocks are stored pre-transposed, so the preload runs
                # on the fast transpose datapath: pt = (mbT_c).T = mb_c
                w = c - 1 if pos_chunk else c
                sl = slice(w * P, (w + 1) * P)
                nc.tensor.matmul(
                    out=pt[:], lhsT=mb[:, sl], rhs=ident,
                    is_transpose=True, start=True, stop=False,
                )

        pd = sb.tile([P, P], F32)
        npos = sb.tile([P, 1], F32)
        e = sb.tile([P, NW], F32)
        r5 = sb.tile([P, nch], F32)
        h = P // B
        for c in order:
            gt = sb.tile([P, P], F32, tag=f"gt{c}")
            nc.gpsimd.indirect_dma_start(
                out=gt[:],
                out_offset=None,
                in_=xt[:],
                in_offset=bass.IndirectOffsetOnAxis(ap=it[:, c : c + 1], axis=0),
            )
            pt = pts[c]
            if pos_chunk and c == 0:
                # chunk 0 rows are the own targets: pos[p] = gt.T[p, p]
                nc.tensor.transpose(out=pt[:], in_=gt[:], identity=ident)
                nc.vector.tensor_tensor(pd[:], pt[:], ident, mybir.AluOpType.mult)
            else:
                # pt += gt.T on the transpose datapath
                nc.tensor.matmul(
                    out=pt[:], lhsT=gt[:], rhs=ident,
                    is_transpose=True, start=False, stop=True,
                )
            if (pos_chunk and c == 0) or (not pos_chunk and c == 4):
                # last diag-carrying chunk landed: finish pos (diag mask mb
                # is 0 there in the 5-chunk variant, so psum diag == gt.T)
                if not pos_chunk:
                    nc.vector.tensor_tensor(
                        pd[0:h, :], pts[2][0:h, :], ident[0:h, :],
                        mybir.AluOpType.mult,
                    )
                    nc.vector.tensor_tensor(
                        pd[h:P, :], pts[4][h:P, :], ident[h:P, :],
                        mybir.AluOpType.mult,
                    )
                nc.vector.reduce_sum(
                    npos[:], pd[:], axis=mybir.AxisListType.X, negate=True
                )
        # fused exp(chunk - pos) with per-row accumulation, read from PSUM;
        # each exp fires as soon as its chunk and npos are ready.  Window
        # chunks 0/1 only hold batch-0 tokens and 3/4 only batch-1 (the
        # other partition half is -1e9-masked), so those exps run half-width.
        nc.vector.memset(r5[:], 0.0)
        for c in order:
            if pos_chunk and c == 0:
                continue
            w = c - 1 if pos_chunk else c
            sl = slice(w * P, (w + 1) * P)
            if w in (0, 1):
                pr = slice(0, h)
            elif w in (3, 4):
                pr = slice(h, P)
            else:
                pr = slice(0, P)
            nc.scalar.activation(
                e[pr, sl], pts[c][pr, :], mybir.ActivationFunctionType.Exp,
                bias=npos[pr, :], scale=1.0, accum_out=r5[pr, c : c + 1],
            )
        r = sb.tile([P, 1], F32)
        if pos_chunk:
            nc.vector.reduce_sum(r[:], r5[:, 1:], axis=mybir.AxisListType.X)
        else:
            nc.vector.reduce_sum(r[:], r5[:], axis=mybir.AxisListType.X)

        # lv[:, 0] = valid * log1p(sum of masked exp):
        #   5-chunk: r already contains the diagonal's exp(0) == 1, so
        #            ln(valid*r + inv) = log1p(.) when valid else ln(1) = 0
        #   6-chunk: ln(valid*r + 1) = log1p(r) when valid else 0
        if pos_chunk:
            nc.scalar.activation(
                lv[:, 0:1], r[:], mybir.ActivationFunctionType.Ln,
                bias=1.0, scale=lv[:, 1:2],
            )
        else:
            nc.scalar.activation(
                lv[:, 0:1], r[:], mybir.ActivationFunctionType.Ln,
                bias=inv[:], scale=lv[:, 1:2],
            )

        # partition reduction: out[2, 1] = lv.T @ ones
        ones = sb.tile([P, 1], F32)
        nc.vector.memset(ones[:], 1.0)
        acc = ps.tile([2, 1], F32, space="PSUM")
        nc.tensor.matmul(out=acc[:], lhsT=lv[:], rhs=ones[:], start=True, stop=True)
        res = sb.tile([2, 1], F32)
        nc.vector.tensor_copy(res[:], acc[:])
        nc.sync.dma_start(out[:], res[:])
    with _one_act_table():
        nc.compile()
    return nc


def _get_nc(pos_chunk):
    key = f"nc{pos_chunk}"
    if key not in _CACHE:
        _CACHE[key] = _build(pos_chunk)
    return _CACHE[key]


def _consts():
    if "consts" not in _CACHE:
        p = np.arange(P, dtype=np.int64)
        il = (p % CI)[:, None]
        bp = (p // CI)[:, None]
        f = np.arange(NW, dtype=np.int64)[None, :]
        jl = f % W
        bf = f // W
        band = (bf == bp) & (jl >= il) & (jl < il + WIN)
        cstv = np.full((P, NW + P), -1e9, np.float32)
        cstv[:, 0:NW][band] = 0.0
        cstv[:, NW:] = np.eye(P, dtype=np.float32)
        # 5-chunk variant: keep each partition's own window-diagonal entry
        # unmasked — its exp term is exactly 1 and plays log1p's "+1"
        cstv5 = cstv.copy()
        p_i = np.arange(P)
        cstv5[p_i, (p_i // CI) * W + WIN + (p_i % CI)] = 0.0
        # store the window mask blocks transposed: the kernel preloads each
        # into PSUM via the transpose datapath (pt = block.T)
        for cv in (cstv, cstv5):
            for c in range(NW // P):
                sl = slice(c * P, (c + 1) * P)
                cv[:, sl] = cv[:, sl].T.copy()
        # diag positions in the flat window: j == i rows per batch
        diagf = np.zeros(NW, bool)
        diagf[WIN : W] = True
        diagf[W + WIN : 2 * W] = True
        _CACHE["consts"] = (
            np.ascontiguousarray(cstv),
            np.ascontiguousarray(cstv5),
            diagf,
        )
    return _CACHE["consts"]


def kernel(input, target, _trace=False):
    input = np.asarray(input, dtype=np.float32)
    target = np.asarray(target)
    cstv6, cstv5, diagf = _consts()
    t32 = target[:, :CT].astype(np.int32)

    # fast path: pos can be read off the window diagonal iff no target in the
    # contrastive range is PAD (0) or negative
    pos_chunk = bool((t32 <= 0).any())

    in_maps = []
    for k in range(NCORES):
        s = k * CI
        lo = s - WIN
        if lo >= 0:
            twk = t32[:, lo : s + CI]
        else:
            twk = np.concatenate(
                [np.zeros((B, -lo), np.int32), t32[:, : s + CI]], axis=1
            )
        tok = t32[:, s : s + CI].reshape(-1)
        win_ids = twk.reshape(-1)
        # tgt_safe clamp + redirect PAD window tokens to the sentinel row V
        # (the own-target/diagonal copies stay clamped so pos is exact)
        safe = np.maximum(win_ids, 0)
        sent_ids = np.where(win_ids == PAD_ID, V, safe)
        if pos_chunk:
            ids_flat = np.concatenate([np.maximum(tok, 0), sent_ids])
        else:
            ids_flat = np.where(diagf, safe, sent_ids)
        nch = ids_flat.size // P
        idxs = np.ascontiguousarray(ids_flat.reshape(nch, P).T)
        xtk = np.empty((V + 1, P), np.float32)
        xtk[:V] = input[:, s : s + CI, :].reshape(P, V).T
        xtk[V:] = -1e9
        in_maps.append(
            {
                "xt": xtk,
                "idx": idxs,
                "to": np.ascontiguousarray(tok).reshape(P, 1),
                "cst": cstv6 if pos_chunk else cstv5,
            }
        )

    nc = _get_nc(pos_chunk)
    br = run_bass_kernel_spmd(
        nc, in_maps, core_ids=list(range(NCORES)), trace=_trace
    )
    rs = np.stack([r["out"] for r in br.results])  # [8, 2, 1]
    loss_sum = rs[:, 0, 0].astype(np.float64).sum()
    cnt = rs[:, 1, 0].astype(np.float64).sum()
    kernel.last_results = br
    return np.asarray(np.float32(loss_sum / max(cnt, 1.0)))


# revision 49
# speedup vs baseline: 1.1388x; 1.0424x over previous
"""ContrastiveTokenLoss on Trainium2 (8 NeuronCores, Bass/Tile).

Problem (hardcoded): input [2, 2048, 32000] f32 logits, target [2, 2048] int.
ct_len = round(2048*0.25) = 512, win = round(512*0.5) = 256,
IGNORE_INDEX = -100, PAD_ID = 0.

loss = sum_{b, i<512} valid(b,i) * log1p( sum_{j in [i-256, i), tgt[b,j]!=0}
           exp(x[b,i,tgt[b,j]] - x[b,i,tgt_safe[b,i]]) ) / max(#valid, 1)

Sharding: the 512 contrastive positions are split across the 8 cores (64 per
core per batch element; positions >= 512 are never touched).  Each core
receives its slab of logits laid out vocab-major ([32001, 128]: the one vocab
row the loss needs per window token is then contiguous; row 32000 is a -1e9
sentinel that PAD window tokens are redirected to, which zeroes their exp
contribution), the per-chunk gather row indices, its own 128 targets, and one
constant table (additive -1e9 band mask | 128x128 identity).

On-device per core: 5 (6 when PAD/ignore tokens are present) indirect DMAs
gather the window rows the loss touches (512 B contiguous each, ~320 KB
instead of the 16 MB slab), PE transposes each [128,128] chunk back to
(position-partition, window-free) layout, DVE adds the band mask straight
out of PSUM and extracts the positive logit from the window diagonal, ACT
computes a fused exp+row-sum with -pos as a per-partition bias, then log1p
and one [128,2]x[128,1] matmul reduce loss / valid-count over partitions.
Each core returns [loss_sum, valid_count]; the host sums the 8 partials and
divides.
"""

import numpy as np
from contextlib import ExitStack

import concourse.bass as bass
import concourse.bacc as bacc
import concourse.mybir as mybir
import concourse.tile as tile
from concourse.bass_utils import run_bass_kernel_spmd

B, T, V = 2, 2048, 32000
CT = 512
WIN = 256
IGNORE_INDEX = -100
PAD_ID = 0
NCORES = 8
CI = CT // NCORES          # 64 positions per core per batch
W = WIN + CI               # 320 window positions per core per batch
NW = B * W                 # 640 window rows per core
P = B * CI                 # 128 partition rows = (batch, local position)
F32 = mybir.dt.float32
I32 = mybir.dt.int32

_CACHE = {}


class _one_act_table:
    """While compiling, restrict Exp/Ln to the combined
    `natural_log_exp_and_others` ACT table set so the kernel needs a single
    table load instead of an Exp-set -> Ln-set swap on the critical path.
    Set ids are positional, so only set CONTENTS are filtered, never order."""

    def __enter__(self):
        self.orig = bacc.get_activation_tables

        def patched(arch):
            t = self.orig(arch)
            both = {
                mybir.ActivationFunctionType.Exp,
                mybir.ActivationFunctionType.Ln,
            }
            return {
                name: (fns if name == "natural_log_exp_and_others" else fns - both)
                for name, fns in t.items()
            }

        bacc.get_activation_tables = patched

    def __exit__(self, *a):
        bacc.get_activation_tables = self.orig


def _build(pos_chunk):
    """pos_chunk=False: 5 gathers, pos read off the window diagonal (exact
    when every target in [0, CT) is > 0, which the host checks).
    pos_chunk=True: an extra leading gather chunk holds the own-target rows
    unsentineled so PAD own-targets still produce the right pos."""
    nch = 6 if pos_chunk else 5
    nc = bacc.Bacc("TRN2", target_bir_lowering=False)
    xt = nc.dram_tensor("xt", [V + 1, P], F32, kind="ExternalInput")
    idx = nc.dram_tensor("idx", [P, nch], I32, kind="ExternalInput")
    to = nc.dram_tensor("to", [P, 1], I32, kind="ExternalInput")
    cst = nc.dram_tensor("cst", [P, NW + P], F32, kind="ExternalInput")
    out = nc.dram_tensor("out", [1, 2], F32, kind="ExternalOutput")

    with ExitStack() as ctx:
        tc = ctx.enter_context(tile.TileContext(nc))
        sb = ctx.enter_context(tc.tile_pool(name="sb", bufs=1))
        ps = ctx.enter_context(tc.tile_pool(name="ps", bufs=1, space="PSUM"))

        # gather row indices first (host pre-clamped and pre-sentineled) so
        # the gathers start as early as possible
        it = sb.tile([P, nch], I32)
        nc.sync.dma_start(it[:], idx[:])

        cst_sb = sb.tile([P, NW + P], F32)
        nc.sync.dma_start(cst_sb[:], cst[:])
        mb = cst_sb[:, 0:NW]
        ident = cst_sb[:, NW : NW + P]
        to_sb = sb.tile([P, 1], I32)
        nc.sync.dma_start(to_sb[:], to[:])

        # valid flags (lv[:, 1]) and, for the ln(valid*r + inv) fold, the
        # inverted flag — both off the critical path
        lv = sb.tile([P, 2], F32)
        nc.vector.tensor_scalar(
            lv[:, 1:2], to_sb[:], IGNORE_INDEX, None, mybir.AluOpType.not_equal
        )
        inv = sb.tile([P, 1], F32)
        nc.vector.tensor_scalar(
            inv[:], to_sb[:], IGNORE_INDEX, None, mybir.AluOpType.is_equal
        )

        # Preload the additive band mask into each window chunk's PSUM bank
        # (psum_c = I.T @ mb_c = mb_c) before the gathers land; the transpose
        # then ACCUMULATES gt.T on top (one matmul, start=False), so exp can
        # read (gt.T + mb) straight from PSUM with no DVE add.  In the
        # 5-chunk variant the mask keeps the window diagonal (own token) at 0
        # so its exp term is exactly 1 == log1p's "+1".
        order = [0, 1, 2, 3, 4, 5] if pos_chunk else [2, 4, 0, 1, 3]
        pts = {}
        for c in order:
            pt = ps.tile([P, P], F32, tag=f"pt{c}", space="PSUM")
            pts[c] = pt
            if not (pos_chunk and c == 0):
                # mask blocks are stored pre-transposed, so the preload runs
                # on the fast transpose datapath: pt = (mbT_c).T = mb_c
                w = c - 1 if pos_chunk else c
                sl = slice(w * P, (w + 1) * P)
                nc.tensor.matmul(
                    out=pt[:], lhsT=mb[:, sl], rhs=ident,
                    is_transpose=True, start=True, stop=False,
                )

        pd = sb.tile([P, P], F32)
        npos = sb.tile([P, 1], F32)
        e = sb.tile([P, NW], F32)
        r5 = sb.tile([P, nch], F32)
        h = P // B
        for c in order:
            gt = sb.tile([P, P], F32, tag=f"gt{c}")
            nc.gpsimd.indirect_dma_start(
                out=gt[:],
                out_offset=None,
                in_=xt[:],
                in_offset=bass.IndirectOffsetOnAxis(ap=it[:, c : c + 1], axis=0),
            )
            pt = pts[c]
            if pos_chunk and c == 0:
                # chunk 0 rows are the own targets: pos[p] = gt.T[p, p]
                nc.tensor.transpose(out=pt[:], in_=gt[:], identity=ident)
                nc.vector.tensor_tensor(pd[:], pt[:], ident, mybir.AluOpType.mult)
            else:
                # pt += gt.T on the transpose datapath
                nc.tensor.matmul(
                    out=pt[:], lhsT=gt[:], rhs=ident,
                    is_transpose=True, start=False, stop=True,
                )
            if (pos_chunk and c == 0) or (not pos_chunk and c == 4):
                # last diag-carrying chunk landed: finish pos (diag mask mb
                # is 0 there in the 5-chunk variant, so psum diag == gt.T)
                if not pos_chunk:
                    nc.vector.tensor_tensor(
                        pd[0:h, :], pts[2][0:h, :], ident[0:h, :],
                        mybir.AluOpType.mult,
                    )
                    nc.vector.tensor_tensor(
                        pd[h:P, :], pts[4][h:P, :], ident[h:P, :],
                        mybir.AluOpType.mult,
                    )
                nc.vector.reduce_sum(
                    npos[:], pd[:], axis=mybir.AxisListType.X, negate=True
                )
        # fused exp(chunk - pos) with per-row accumulation, read from PSUM;
        # each exp fires as soon as its chunk and npos are ready.  Window
        # chunks 0/1 only hold batch-0 tokens and 3/4 only batch-1 (the
        # other partition half is -1e9-masked), so those exps run half-width.
        nc.vector.memset(r5[:], 0.0)
        for c in order:
            if pos_chunk and c == 0:
                continue
            w = c - 1 if pos_chunk else c
            sl = slice(w * P, (w + 1) * P)
            if w in (0, 1):
                pr = slice(0, h)
            elif w in (3, 4):
                pr = slice(h, P)
            else:
                pr = slice(0, P)
            nc.scalar.activation(
                e[pr, sl], pts[c][pr, :], mybir.ActivationFunctionType.Exp,
                bias=npos[pr, :], scale=1.0, accum_out=r5[pr, c : c + 1],
            )
        r = sb.tile([P, 1], F32)
        if pos_chunk:
            nc.vector.reduce_sum(r[:], r5[:, 1:], axis=mybir.AxisListType.X)
        else:
            nc.vector.reduce_sum(r[:], r5[:], axis=mybir.AxisListType.X)

        # lv[:, 0] = valid * log1p(sum of masked exp):
        #   5-chunk: r already contains the diagonal's exp(0) == 1, so
        #            ln(valid*r + inv) = log1p(.) when valid else ln(1) = 0
        #   6-chunk: ln(valid*r + 1) = log1p(r) when valid else 0
        if pos_chunk:
            nc.scalar.activation(
                lv[:, 0:1], r[:], mybir.ActivationFunctionType.Ln,
                bias=1.0, scale=lv[:, 1:2],
            )
        else:
            nc.scalar.activation(
                lv[:, 0:1], r[:], mybir.ActivationFunctionType.Ln,
                bias=inv[:], scale=lv[:, 1:2],
            )

        # partition reduction on the (idle) gpsimd engine, axis=C: one hop
        # instead of Ln -> PE matmul -> PSUM copy -> DMA
        res = sb.tile([1, 2], F32)
        nc.gpsimd.reduce_sum(res[:], lv[:], axis=mybir.AxisListType.C)
        nc.sync.dma_start(out[:], res[:])
    with _one_act_table():
        nc.compile()
    return nc


def _get_nc(pos_chunk):
    key = f"nc{pos_chunk}"
    if key not in _CACHE:
        _CACHE[key] = _build(pos_chunk)
    return _CACHE[key]


def _consts():
    if "consts" not in _CACHE:
        p = np.arange(P, dtype=np.int64)
        il = (p % CI)[:, None]
        bp = (p // CI)[:, None]
        f = np.arange(NW, dtype=np.int64)[None, :]
        jl = f % W
        bf = f // W
        band = (bf == bp) & (jl >= il) & (jl < il + WIN)
        cstv = np.full((P, NW + P), -1e9, np.float32)
        cstv[:, 0:NW][band] = 0.0
        cstv[:, NW:] = np.eye(P, dtype=np.float32)
        # 5-chunk variant: keep each partition's own window-diagonal entry
        # unmasked — its exp term is exactly 1 and plays log1p's "+1"
        cstv5 = cstv.copy()
        p_i = np.arange(P)
        cstv5[p_i, (p_i // CI) * W + WIN + (p_i % CI)] = 0.0
        # store the window mask blocks transposed: the kernel preloads each
        # into PSUM via the transpose datapath (pt = block.T)
        for cv in (cstv, cstv5):
            for c in range(NW // P):
                sl = slice(c * P, (c + 1) * P)
                cv[:, sl] = cv[:, sl].T.copy()
        # diag positions in the flat window: j == i rows per batch
        diagf = np.zeros(NW, bool)
        diagf[WIN : W] = True
        diagf[W + WIN : 2 * W] = True
        _CACHE["consts"] = (
            np.ascontiguousarray(cstv),
            np.ascontiguousarray(cstv5),
            diagf,
        )
    return _CACHE["consts"]


def kernel(input, target, _trace=False):
    input = np.asarray(input, dtype=np.float32)
    target = np.asarray(target)
    cstv6, cstv5, diagf = _consts()
    t32 = target[:, :CT].astype(np.int32)

    # fast path: pos can be read off the window diagonal iff no target in the
    # contrastive range is PAD (0) or negative
    pos_chunk = bool((t32 <= 0).any())

    in_maps = []
    for k in range(NCORES):
        s = k * CI
        lo = s - WIN
        if lo >= 0:
            twk = t32[:, lo : s + CI]
        else:
            twk = np.concatenate(
                [np.zeros((B, -lo), np.int32), t32[:, : s + CI]], axis=1
            )
        tok = t32[:, s : s + CI].reshape(-1)
        win_ids = twk.reshape(-1)
        # tgt_safe clamp + redirect PAD window tokens to the sentinel row V
        # (the own-target/diagonal copies stay clamped so pos is exact)
        safe = np.maximum(win_ids, 0)
        sent_ids = np.where(win_ids == PAD_ID, V, safe)
        if pos_chunk:
            ids_flat = np.concatenate([np.maximum(tok, 0), sent_ids])
        else:
            ids_flat = np.where(diagf, safe, sent_ids)
        nch = ids_flat.size // P
        idxs = np.ascontiguousarray(ids_flat.reshape(nch, P).T)
        xtk = np.empty((V + 1, P), np.float32)
        xtk[:V] = input[:, s : s + CI, :].reshape(P, V).T
        xtk[V:] = -1e9
        in_maps.append(
            {
                "xt": xtk,
                "idx": idxs,
                "to": np.ascontiguousarray(tok).reshape(P, 1),
                "cst": cstv6 if pos_chunk else cstv5,
            }
        )

    nc = _get_nc(pos_chunk)
    br = run_bass_kernel_spmd(
        nc, in_maps, core_ids=list(range(NCORES)), trace=_trace
    )
    rs = np.stack([r["out"][0] for r in br.results])  # [8, 2]
    loss_sum = rs[:, 0].astype(np.float64).sum()
    cnt = rs[:, 1].astype(np.float64).sum()
    kernel.last_results = br
    return np.asarray(np.float32(loss_sum / max(cnt, 1.0)))
